# revision 37
# baseline (speedup 1.0000x reference)
"""Distributed brute-force kNN (top-50 inverse-distance-weighted regression).

Strategy (8 NeuronCores):
  - Table (500k x 64) is norm-sorted, striped across 8 cores (62500 rows each,
    padded to 65536) and laid out so that every 8 consecutive-norm rows
    ("octet") land at the same free-dim position j across the 8 matmul tiles
    of a superchunk.
  - Each core holds its bf16 transposed table slice resident in SBUF and,
    per 128-query group, runs 128 matmuls (K=64, N=512) producing q.t scores
    in PSUM, compressed on the fly by a 7-op tensor_max tournament tree into
    per-octet maxima of q.t (group max <=> group min of distance, since octet
    norms are nearly equal).
  - Cores emit [1024, 8192] bf16 octet-maxima; host ranks all 65536 octets
    per query by a distance lower bound (min_octet_norm - 2*gmax), takes the
    top N_GROUPS as candidates (huge margin over bf16 noise), re-scores the
    8*N_GROUPS candidate rows exactly in f32 with the reference formula, and
    does the final top-50 + inverse-distance weighting.
"""

import numpy as np
import ml_dtypes

import concourse.bass as bass
import concourse.tile as tile
from concourse import mybir
from concourse.bass_utils import run_bass_kernel_spmd
from contextlib import ExitStack

# Problem geometry (hardcoded per spec).
B = 1024          # queries
D = 64            # feature dim
C = 500000        # table capacity
K = 50            # neighbours
DELTA = 1e-3

CORES = 8
C_LOC = 65536     # padded per-core capacity
N_TILE = 512      # matmul free dim (one PSUM bank)
G = 8             # octet size == tiles per superchunk
SC = C_LOC // (G * N_TILE)   # 16 superchunks
GM_W = C_LOC // G            # 8192 octet maxima per query per core
P = 128           # partition dim == query-group size
QG = B // P       # 8 query groups

N_GROUPS = 96     # candidate octets per query taken on host (margin over top-50)

_NC_CACHE = {}


def _build_nc():
    """Bass program for one core: resident-table scan + octet-max compression."""
    if "nc" in _NC_CACHE:
        return _NC_CACHE["nc"]
    nc = bass.Bass()
    # Table and queries share one dram tensor so the resident SBUF copy
    # arrives via a single DMA stream (single wait source for matmuls).
    tq = nc.declare_dram_parameter("tq", [D, C_LOC + B], mybir.dt.bfloat16,
                                   isOutput=False)
    gm = nc.declare_dram_parameter("gm", [B, GM_W], mybir.dt.bfloat16, isOutput=True)

    TRIP = 2                           # matmul tiles batched per TensorReduce
    n_trips = 62                       # 62 * 1024 = 63488 >= 62500 real rows
    oct_per_trip = TRIP * N_TILE // G  # 128 octet maxima per pair
    DUMMY_IVL = 2                      # absorber every 2 pairs
    last_col = n_trips * oct_per_trip - 1
    pad_col = GM_W - 1                 # sync-pad column (never TR'd)

    from concourse.bass import _add_dep_helper as dep
    # Collapse all SWDGE DMAs onto one queue/proc: fewer distinct DMA sems
    # keeps the end-of-kernel drain within the per-inst sync-wait budget.
    import concourse.tile_sem_assignment as _tsa
    _tsa.NUM_SWDGE_GLOBAL_SEMS = 1

    # This walrus build rejects instructions with more than one sem wait, but
    # TileContext's exit path emits a single drain waiting on every active
    # proc. Split it into one drain per proc, each carrying one wait.
    from concourse.vector_clock import VectorClock, ScopedClock

    def _split_drain_and_barrier(self, tick_clock, wait_clock):
        gc = tick_clock.global_clock
        for proc in range(27):
            t = gc.peek_next(proc) - 1
            if t <= 0:
                continue
            d = self.nc.sync.drain()
            pc = VectorClock()
            pc.require_at_least(proc, t)
            wait_clock.add_sem_waits(d.ins, ScopedClock({None: pc}))
        self.nc.all_engine_barrier()
        assert self.sems is not None
        popped = self.nc._tile_sem_poison_stack.pop()
        assert popped is self._sem_poison
        self.nc.clear_and_free_semaphores(list(self.sems.allocated().values()))
        self.nc.all_engine_barrier()

    tile.TileContext._drain_and_barrier = _split_drain_and_barrier

    # Wait budget: this walrus build accepts at most ONE semaphore wait per
    # compute instruction. Engines do not implicitly observe even their own
    # completions, so every hazard needs a sem wait on the consuming engine's
    # stream -- but once an engine has waited on a (sem, tick), later deps at
    # lower ticks are elided. The absorber ops below each carry exactly one
    # wait so the worker ops (matmul / tensor_reduce) keep exactly one:
    #   - ldweights D(j): reads a DVE-written gm column -> pulls DVE progress
    #     into the PE stream (its weight load is clobbered by the next
    #     matmul's own ldweights, so it has no output at all).
    #   - A1: DVE copy reading the newest gm column of qg-1 -> observes all
    #     prior DVE ticks.  A2: DVE copy writing the sync-pad column ->
    #     carries the out-DMA(qg-2) WAR.
    with ExitStack() as ctx:
        tc = ctx.enter_context(tile.TileContext(nc))
        singles = ctx.enter_context(tc.tile_pool(name="singles", bufs=1))
        psum = ctx.enter_context(tc.tile_pool(name="psum", bufs=3, space="PSUM"))
        dpsum = ctx.enter_context(tc.tile_pool(name="dpsum", bufs=2, space="PSUM"))
        gmpool = ctx.enter_context(tc.tile_pool(name="gmpool", bufs=2))
        apool = ctx.enter_context(tc.tile_pool(name="apool", bufs=1))

        # table load on the SWDGE path so the HWDGE out-DMA queues never
        # wrap onto its queue (which would add a second sem wait there)
        t_sb = singles.tile([D, C_LOC + B], mybir.dt.bfloat16)
        nc.gpsimd.dma_start(out=t_sb, in_=tq[:, :])

        a1_scr = apool.tile([1, 1], mybir.dt.bfloat16, tag="a1")
        prev_gms = []
        chain = {"PE": None, "DVE": None}

        def link(key, inst):
            # pin a total order per engine queue so the scheduler cannot
            # reorder around the absorbers (which would form cycles against
            # the PSUM bank-reuse edges).  add_dep_helper(a, b) == "a waits
            # on b", so the new inst waits on the chain tail.
            if chain[key] is not None:
                dep(inst.ins, chain[key].ins, sync=False, reason=f"{key} order")
            chain[key] = inst
            return inst

        for qg in range(QG):
            lhsT = t_sb[:, C_LOC + qg * P: C_LOC + (qg + 1) * P]
            gm_tile = gmpool.tile([P, GM_W], mybir.dt.bfloat16)
            if qg >= 1:
                pg = prev_gms[-1]
                link("DVE", nc.vector.tensor_copy(
                    a1_scr, pg[0:1, last_col:last_col + 1]))
                if qg >= 2:
                    link("DVE", nc.vector.tensor_copy(
                        gm_tile[0:1, pad_col:pad_col + 1], pg[0:1, 0:1]))
            for t in range(n_trips):
                if t % DUMMY_IVL == 0:
                    # read the newest gm column the PSUM-reuse WAR actually
                    # requires (TR of t-2, bufs=2) so PE leads DVE by two
                    # triples instead of ping-ponging behind TR(t-1)
                    if t >= 2:
                        col = (t - 1) * oct_per_trip - 1
                        dsrc = gm_tile[0:D, col:col + 1]
                    elif qg > 0:
                        dsrc = prev_gms[-1][0:D, last_col:last_col + 1]
                    else:
                        dsrc = t_sb[:, 0:1]
                    ddst = dpsum.tile([1, 1], mybir.dt.float32, tag="dp")
                    link("PE", nc.tensor.matmul(ddst, t_sb[:, 0:1], dsrc[0:D, :],
                                                start=True, stop=True))
                p = psum.tile([P, TRIP * N_TILE], mybir.dt.float32, tag="ps")
                for j in range(TRIP):
                    c0 = (t * TRIP + j) * N_TILE
                    link("PE", nc.tensor.matmul(
                        p[:, j * N_TILE:(j + 1) * N_TILE], lhsT,
                        t_sb[:, c0:c0 + N_TILE],
                        start=True, stop=True))
                # octet max straight out of PSUM: [128, 192, 8] -max-> [128, 192]
                link("DVE", nc.vector.tensor_reduce(
                    out=gm_tile[:, t * oct_per_trip:(t + 1) * oct_per_trip],
                    in_=p.rearrange("p (o g) -> p o g", g=G),
                    axis=mybir.AxisListType.X,
                    op=mybir.AluOpType.max,
                ))
            nc.sync.dma_start(out=gm[qg * P:(qg + 1) * P, :], in_=gm_tile)
            prev_gms.append(gm_tile)

    _NC_CACHE["nc"] = nc
    return nc


def _preprocess(table_keys):
    """Norm-sort, stripe across cores, octet layout. Returns per-core device
    tables plus the index/norm maps needed on the way back."""
    tk = np.ascontiguousarray(table_keys, dtype=np.float32)
    norms = np.einsum("ij,ij->i", tk, tk)
    order = np.argsort(norms, kind="stable")

    tts = []                       # [CORES] of [D, C_LOC] bf16
    cand_rows = np.full((CORES, GM_W, G), -1, dtype=np.int64)
    gnorm_min = np.full((CORES, GM_W), np.float32(1e9), dtype=np.float32)

    n_loc = C // CORES             # 62500 real rows per core
    for m in range(CORES):
        rows_m = order[m::CORES]                    # local norm-ascending rows
        T = np.zeros((C_LOC, D), dtype=np.float32)  # padded, position==rank
        T[:n_loc] = tk[rows_m]
        nrm = np.full(C_LOC, np.float32(1e9), dtype=np.float32)
        nrm[:n_loc] = norms[rows_m]

        # octet o covers device columns (== local norm ranks) [8o, 8o+8)
        flat_groups = np.arange(C_LOC).reshape(GM_W, G)
        valid = flat_groups < n_loc
        cand_rows[m] = np.where(valid, rows_m[np.minimum(flat_groups, n_loc - 1)], -1)
        gnorm_min[m] = np.where(valid[:, 0], nrm[flat_groups[:, 0]],
                                np.float32(1e9))

        tts.append(np.ascontiguousarray(T.T.astype(ml_dtypes.bfloat16)))

    return tts, cand_rows, gnorm_min


def kernel(keys, table_keys, table_values):
    q = np.ascontiguousarray(keys, dtype=np.float32)
    tk = np.ascontiguousarray(table_keys, dtype=np.float32)
    v = np.ascontiguousarray(table_values, dtype=np.float32)

    tts, cand_rows, gnorm_min = _preprocess(tk)
    qt = q.T.astype(ml_dtypes.bfloat16)
    tqs = [np.ascontiguousarray(np.concatenate([tts[m], qt], axis=1))
           for m in range(CORES)]

    nc = _build_nc()
    in_maps = [{"tq": tqs[m]} for m in range(CORES)]
    res = run_bass_kernel_spmd(nc, in_maps, core_ids=list(range(CORES)))
    gmax = np.stack([r["gm"].astype(np.float32) for r in res.results])  # [8, B, GM_W]

    # ---- host stage 2: rank octets by distance lower bound ----
    # Pad octets (incl. the never-written sync-pad tile) carry uninitialized
    # SBUF garbage in gmax; neutralize them before ranking.
    invalid_g = gnorm_min >= np.float32(1e9)         # [8, GM_W]
    gmax = np.where(invalid_g[:, None, :], np.float32(-1e9),
                    np.nan_to_num(gmax, nan=-1e9, posinf=-1e9, neginf=-1e9))
    # lb_g = min_norm_g - 2 * gmax_g  (|q|^2 omitted: constant per query)
    lb = gnorm_min[:, None, :] - 2.0 * gmax          # [8, B, GM_W]
    lb = lb.transpose(1, 0, 2).reshape(B, CORES * GM_W)
    top_g = np.argpartition(lb, N_GROUPS, axis=1)[:, :N_GROUPS]   # [B, N_GROUPS]

    # expand candidate groups -> member rows
    core_of = top_g // GM_W
    g_of = top_g % GM_W
    rows = cand_rows[core_of, g_of]                   # [B, N_GROUPS, G]
    rows = rows.reshape(B, N_GROUPS * G)              # [B, NCAND]
    invalid = rows < 0
    rows_safe = np.where(invalid, 0, rows)

    # ---- exact rescore with the reference's formula (f32) ----
    tc_ = tk[rows_safe]                               # [B, NCAND, D]
    qn = np.einsum("ij,ij->i", q, q)                  # |q|^2
    tn = np.einsum("ij,ij->i", tk, tk)[rows_safe]     # |t|^2
    dots = np.einsum("bd,bkd->bk", q, tc_)
    d2 = qn[:, None] - 2.0 * dots + tn
    d2 = np.where(invalid, np.float32(np.inf), d2).astype(np.float32)

    top_k = np.argpartition(d2, K, axis=1)[:, :K]     # [B, K]
    rows_k = np.take_along_axis(rows_safe, top_k, axis=1)

    # ---- reference tail: exact sq, inverse-distance weights ----
    nb = tk[rows_k]                                   # [B, K, D]
    sq = np.sum((q[:, None, :] - nb) ** 2, axis=2, dtype=np.float32)
    w = np.float32(1.0) / (sq + np.float32(DELTA))
    w = w / np.sum(w, axis=1, keepdims=True)
    out = np.sum(w * v[rows_k], axis=1)
    return out.astype(np.float32)


# revision 46
# speedup vs baseline: 1.2473x; 1.2473x over previous
"""Distributed brute-force kNN (top-50 inverse-distance-weighted regression).

Strategy (8 NeuronCores):
  - Table (500k x 64) is norm-sorted and striped across 8 cores; each core's
    62500 rows (padded to 63488) are laid out so every 8 consecutive-norm
    rows (an "octet") map to one output column of a 1024-column compression
    unit ("pair" of two K=64 N=512 bf16 matmuls into one 2-bank PSUM tile).
  - Per 128-query group, each unit's 1024 q.t scores are reduced to 128
    octet maxima by one of two paths, balanced across the assist engines:
      * E-units (46/62): ScalarE evicts PSUM to bf16 SBUF, DVE folds it
        3x with tensor_max halves (stride-128 octets).
      * R-units (16/62): DVE tensor_reduce(max) straight from PSUM
        (consecutive octets).
  - Cores emit [1024, 8192] bf16 octet maxima; host ranks all octets per
    query by a distance lower bound (min_octet_norm - 2*gmax), takes the top
    N_GROUPS candidates (margin >> bf16 noise), re-scores the 8*N_GROUPS
    candidate rows exactly in f32 with the reference formula, and does the
    final top-50 + inverse-distance weighting.

This walrus build accepts at most ONE semaphore wait per instruction, and
Tile emits a wait per hazard with no transitive clock tracking, so the
builder threads absorber ops (standalone ldweights on PE, tiny copies on
ACT/DVE) and total per-engine ordering through the pipeline.
"""

import numpy as np
import ml_dtypes

import concourse.bass as bass
import concourse.tile as tile
from concourse import mybir
from concourse.bass_utils import run_bass_kernel_spmd
from contextlib import ExitStack

# Problem geometry (hardcoded per spec).
B = 1024          # queries
D = 64            # feature dim
C = 500000        # table capacity
K = 50            # neighbours
DELTA = 1e-3

CORES = 8
N_TILE = 512      # matmul free dim (one PSUM bank)
PAIR_W = 2 * N_TILE
N_UNITS = 62      # 62 * 1024 = 63488 >= 62500 real rows per core
N_COLS = N_UNITS * PAIR_W
C_LOC = 65536     # t_sb table width (first N_COLS used)
G = 8             # octet size
GM_W = 8192       # gm row width (62*128 = 7936 real octet columns + pad)
P = 128           # partition dim == query-group size
QG = B // P       # 8 query groups

N_GROUPS = 96     # candidate octets per query taken on host

# unit kinds per query group: blocks of [E,E,E,R] -> 46 E + 16 R
UNITS = ([("E")] * 0)
UNITS = []
for i in range(15):
    UNITS += ["E", "E", "E", "R"]
UNITS += ["E", "R"]
assert len(UNITS) == N_UNITS and UNITS.count("E") == 46

_NC_CACHE = {}


def _build_nc():
    if "nc" in _NC_CACHE:
        return _NC_CACHE["nc"]
    nc = bass.Bass()
    tq = nc.declare_dram_parameter("tq", [D, C_LOC + B], mybir.dt.bfloat16,
                                   isOutput=False)
    gm = nc.declare_dram_parameter("gm", [B, GM_W], mybir.dt.bfloat16, isOutput=True)

    OCT_W = PAIR_W // G            # 128 octet columns per unit
    last_col = N_UNITS * OCT_W - 1
    pad_col = GM_W - 1             # sync-pad column (never written by folds/TR)

    from concourse.bass import _add_dep_helper as dep
    import concourse.tile_sem_assignment as _tsa
    _tsa.NUM_SWDGE_GLOBAL_SEMS = 1

    # Split TileContext's exit drain (one wait per active proc) into
    # one-wait-per-drain instructions.
    from concourse.vector_clock import VectorClock, ScopedClock

    def _split_drain_and_barrier(self, tick_clock, wait_clock):
        gc = tick_clock.global_clock
        for proc in range(27):
            t = gc.peek_next(proc) - 1
            if t <= 0:
                continue
            d = self.nc.sync.drain()
            pc = VectorClock()
            pc.require_at_least(proc, t)
            wait_clock.add_sem_waits(d.ins, ScopedClock({None: pc}))
        self.nc.all_engine_barrier()
        assert self.sems is not None
        popped = self.nc._tile_sem_poison_stack.pop()
        assert popped is self._sem_poison
        self.nc.clear_and_free_semaphores(list(self.sems.allocated().values()))
        self.nc.all_engine_barrier()

    tile.TileContext._drain_and_barrier = _split_drain_and_barrier

    with ExitStack() as ctx:
        tc = ctx.enter_context(tile.TileContext(nc, pool_alloc_mode="queue"))
        singles = ctx.enter_context(tc.tile_pool(name="singles", bufs=1))
        # separate PSUM pools per consumer engine: within a tag all releases
        # come from one engine in issue order, so slot reuse is deterministic
        ppoolE = ctx.enter_context(tc.tile_pool(name="ppoolE", bufs=3, space="PSUM"))
        ppoolR = ctx.enter_context(tc.tile_pool(name="ppoolR", bufs=1, space="PSUM"))
        spool = ctx.enter_context(tc.tile_pool(name="spool", bufs=8))
        m1pool = ctx.enter_context(tc.tile_pool(name="m1pool", bufs=4))
        m2pool = ctx.enter_context(tc.tile_pool(name="m2pool", bufs=4))
        gmpool = ctx.enter_context(tc.tile_pool(name="gmpool", bufs=2))
        apool = ctx.enter_context(tc.tile_pool(name="apool", bufs=2))

        t_sb = singles.tile([D, C_LOC + B], mybir.dt.bfloat16)
        nc.gpsimd.dma_start(out=t_sb, in_=tq[:, :])

        chain = {"PE": None, "DVE": None, "ACT": None}

        def link(key, inst):
            # total order per engine queue (add_dep_helper(a, b) == a waits b)
            if chain[key] is not None:
                dep(inst.ins, chain[key].ins, sync=False, reason=f"{key} order")
            chain[key] = inst
            return inst

        a1_scr = apool.tile([1, 1], mybir.dt.bfloat16, tag="a1")
        tA1a = apool.tile([1, 1], mybir.dt.bfloat16, tag="tA1a")
        tA1b = apool.tile([1, 1], mybir.dt.bfloat16, tag="tA1b")
        tA2 = apool.tile([1, 1], mybir.dt.bfloat16, tag="tA2")
        tA1s = [tA1a, tA1b]

        prev_gms = []
        e_srcs = []            # [64,1] s-tile AP per E-unit (absorber source)
        r_srcs = []            # [64,1] gm-col AP per R-unit
        e_gm_cols = []         # (gm_tile, col) per E-unit, for ACT absorbers
        n_e = 0                # global E-unit counter
        n_r = 0                # global R-unit counter

        for qg in range(QG):
            lhsT = t_sb[:, C_LOC + qg * P: C_LOC + (qg + 1) * P]
            gm_tile = gmpool.tile([P, GM_W], mybir.dt.bfloat16)
            if qg >= 1:
                pg = prev_gms[-1]
                # DVE qg-boundary absorbers: A1 observes all prior DVE ticks,
                # A2 carries the out-DMA(qg-2) WAR via the sync-pad column.
                link("DVE", nc.vector.tensor_copy(
                    a1_scr, pg[0:1, last_col:last_col + 1]))
                if qg >= 2:
                    link("DVE", nc.vector.tensor_copy(
                        gm_tile[0:1, pad_col:pad_col + 1], pg[0:1, 0:1]))

            for u, kind in enumerate(UNITS):
                tbase = u * PAIR_W
                gbase = u * OCT_W
                if kind == "E" and n_e % 4 == 0 and n_e >= 4:
                    # ACT absorbers: tA1 <- read(old E-unit gm col) carries the
                    # DVE wait (covers s-slot fold reads); tA2 <- read(tA1)
                    # carries the ACT-self wait at the newest tick.
                    g_old, c_old = e_gm_cols[n_e - 4]
                    tA1 = tA1s[(n_e // 4) % 2]
                    link("ACT", nc.scalar.copy(tA1, g_old[0:1, c_old:c_old + 1]))
                    link("ACT", nc.scalar.copy(tA2, tA1))

                # PE absorber: standalone ldweights (no output; clobbered by
                # the next matmul's weight load) reading the output of the
                # consumer that released this pair's PSUM slot.
                if kind == "E":
                    dsrc = e_srcs[n_e - 3] if n_e >= 3 else t_sb[0:D, 0:1]
                else:
                    dsrc = r_srcs[n_r - 1] if n_r >= 1 else t_sb[0:D, 0:1]
                link("PE", nc.tensor.ldweights(weights=dsrc))

                pool_, tag_ = (ppoolE, "ppE") if kind == "E" else (ppoolR, "ppR")
                p = pool_.tile([P, PAIR_W], mybir.dt.float32, tag=tag_)
                for j in range(2):
                    c0 = tbase + j * N_TILE
                    link("PE", nc.tensor.matmul(
                        p[:, j * N_TILE:(j + 1) * N_TILE], lhsT,
                        t_sb[:, c0:c0 + N_TILE], start=True, stop=True))

                if kind == "E":
                    s = spool.tile([P, PAIR_W], mybir.dt.bfloat16, tag="s")
                    link("ACT", nc.scalar.copy(s, p))
                    e_srcs.append(s[0:D, 0:1])
                    m1 = m1pool.tile([P, 512], mybir.dt.bfloat16, tag="m1")
                    link("DVE", nc.vector.tensor_max(m1, s[:, 0:512], s[:, 512:1024]))
                    m2 = m2pool.tile([P, 256], mybir.dt.bfloat16, tag="m2")
                    link("DVE", nc.vector.tensor_max(m2, m1[:, 0:256], m1[:, 256:512]))
                    link("DVE", nc.vector.tensor_max(
                        gm_tile[:, gbase:gbase + OCT_W],
                        m2[:, 0:128], m2[:, 128:256]))
                    e_gm_cols.append((gm_tile, gbase + OCT_W - 1))
                    n_e += 1
                else:
                    link("DVE", nc.vector.tensor_reduce(
                        out=gm_tile[:, gbase:gbase + OCT_W],
                        in_=p.rearrange("p (o g) -> p o g", g=G),
                        axis=mybir.AxisListType.X,
                        op=mybir.AluOpType.max,
                    ))
                    r_srcs.append(
                        gm_tile[0:D, gbase + OCT_W - 1:gbase + OCT_W])
                    n_r += 1
            nc.sync.dma_start(out=gm[qg * P:(qg + 1) * P, :], in_=gm_tile)
            prev_gms.append(gm_tile)

    _NC_CACHE["nc"] = nc
    return nc


def _preprocess(table_keys):
    """Norm-sort, stripe across cores, per-unit octet layout.
    Octet o always covers local norm ranks [8o, 8o+8); only the device
    column placement differs per unit kind (E: transposed, R: identity)."""
    tk = np.ascontiguousarray(table_keys, dtype=np.float32)
    norms = np.einsum("ij,ij->i", tk, tk)
    order = np.argsort(norms, kind="stable")

    OCT_W = PAIR_W // G
    tts = []
    cand_rows = np.full((CORES, GM_W, G), -1, dtype=np.int64)
    gnorm_min = np.full((CORES, GM_W), np.float32(1e9), dtype=np.float32)

    n_loc = C // CORES             # 62500 real rows per core
    for m in range(CORES):
        rows_m = order[m::CORES]
        Ts = np.zeros((N_COLS, D), dtype=np.float32)      # rank-indexed
        Ts[:n_loc] = tk[rows_m]
        nrm = np.full(N_COLS, np.float32(1e9), dtype=np.float32)
        nrm[:n_loc] = norms[rows_m]

        Tdev = np.zeros((C_LOC, D), dtype=np.float32)     # position-indexed
        for u, kind in enumerate(UNITS):
            base = u * PAIR_W
            blk = Ts[base:base + PAIR_W]
            if kind == "E":
                # rank r = base + j*8 + k  ->  col = base + k*128 + j
                Tdev[base:base + PAIR_W] = (
                    blk.reshape(OCT_W, G, D).transpose(1, 0, 2).reshape(PAIR_W, D))
            else:
                Tdev[base:base + PAIR_W] = blk

        j = np.arange(GM_W)
        ranks = j[:, None] * G + np.arange(G)[None, :]    # octet o -> ranks
        valid = (ranks < n_loc) & (ranks < N_COLS)
        ranks_c = np.minimum(ranks, n_loc - 1)
        cand_rows[m] = np.where(valid, rows_m[ranks_c], -1)
        gnorm_min[m] = np.where(valid[:, 0],
                                nrm[np.minimum(ranks[:, 0], N_COLS - 1)],
                                np.float32(1e9))

        tts.append(np.ascontiguousarray(Tdev.T.astype(ml_dtypes.bfloat16)))

    return tts, cand_rows, gnorm_min


def kernel(keys, table_keys, table_values):
    q = np.ascontiguousarray(keys, dtype=np.float32)
    tk = np.ascontiguousarray(table_keys, dtype=np.float32)
    v = np.ascontiguousarray(table_values, dtype=np.float32)

    tts, cand_rows, gnorm_min = _preprocess(tk)
    qt = q.T.astype(ml_dtypes.bfloat16)
    tqs = [np.ascontiguousarray(np.concatenate([tts[m], qt], axis=1))
           for m in range(CORES)]

    nc = _build_nc()
    in_maps = [{"tq": tqs[m]} for m in range(CORES)]
    res = run_bass_kernel_spmd(nc, in_maps, core_ids=list(range(CORES)))
    gmax = np.stack([r["gm"].astype(np.float32) for r in res.results])  # [8, B, GM_W]

    # ---- host stage 2: rank octets by distance lower bound ----
    invalid_g = gnorm_min >= np.float32(1e9)         # [8, GM_W]
    gmax = np.where(invalid_g[:, None, :], np.float32(-1e9),
                    np.nan_to_num(gmax, nan=-1e9, posinf=-1e9, neginf=-1e9))
    lb = gnorm_min[:, None, :] - 2.0 * gmax          # [8, B, GM_W]
    lb = lb.transpose(1, 0, 2).reshape(B, CORES * GM_W)
    top_g = np.argpartition(lb, N_GROUPS, axis=1)[:, :N_GROUPS]

    core_of = top_g // GM_W
    g_of = top_g % GM_W
    rows = cand_rows[core_of, g_of].reshape(B, N_GROUPS * G)
    invalid = rows < 0
    rows_safe = np.where(invalid, 0, rows)

    # ---- exact rescore with the reference's formula (f32) ----
    tc_ = tk[rows_safe]                               # [B, NCAND, D]
    qn = np.einsum("ij,ij->i", q, q)
    tn = np.einsum("ij,ij->i", tk, tk)[rows_safe]
    dots = np.einsum("bd,bkd->bk", q, tc_)
    d2 = qn[:, None] - 2.0 * dots + tn
    d2 = np.where(invalid, np.float32(np.inf), d2).astype(np.float32)

    top_k = np.argpartition(d2, K, axis=1)[:, :K]
    rows_k = np.take_along_axis(rows_safe, top_k, axis=1)

    # ---- reference tail: exact sq, inverse-distance weights ----
    nb = tk[rows_k]
    sq = np.sum((q[:, None, :] - nb) ** 2, axis=2, dtype=np.float32)
    w = np.float32(1.0) / (sq + np.float32(DELTA))
    w = w / np.sum(w, axis=1, keepdims=True)
    out = np.sum(w * v[rows_k], axis=1)
    return out.astype(np.float32)


# revision 56
# speedup vs baseline: 1.3284x; 1.0650x over previous
"""Distributed brute-force kNN (top-50 inverse-distance-weighted regression).

Strategy (8 NeuronCores):
  - Table (500k x 64) is norm-sorted and striped across 8 cores; each core's
    62500 rows (padded to 63488) are laid out so every 8 consecutive-norm
    rows (an "octet") map to one output column of a 1024-column compression
    unit ("pair" of two K=64 N=512 bf16 matmuls into one 2-bank PSUM tile).
  - Per 128-query group, each unit's 1024 q.t scores are reduced to 128
    octet maxima by one of two paths, balanced across the assist engines:
      * E-units (46/62): ScalarE evicts PSUM to bf16 SBUF, DVE folds it
        3x with tensor_max halves (stride-128 octets).
      * R-units (16/62): DVE tensor_reduce(max) straight from PSUM
        (consecutive octets).
  - Cores emit [1024, 8192] bf16 octet maxima; host ranks all octets per
    query by a distance lower bound (min_octet_norm - 2*gmax), takes the top
    N_GROUPS candidates (margin >> bf16 noise), re-scores the 8*N_GROUPS
    candidate rows exactly in f32 with the reference formula, and does the
    final top-50 + inverse-distance weighting.

This walrus build accepts at most ONE semaphore wait per instruction, and
Tile emits a wait per hazard with no transitive clock tracking, so the
builder threads absorber ops (standalone ldweights on PE, tiny copies on
ACT/DVE) and total per-engine ordering through the pipeline.
"""

import numpy as np
import ml_dtypes

import concourse.bass as bass
import concourse.tile as tile
from concourse import mybir
from concourse.bass_utils import run_bass_kernel_spmd
from contextlib import ExitStack

# Problem geometry (hardcoded per spec).
B = 1024          # queries
D = 64            # feature dim
C = 500000        # table capacity
K = 50            # neighbours
DELTA = 1e-3

CORES = 8
N_TILE = 512      # matmul free dim (one PSUM bank)
PAIR_W = 2 * N_TILE
N_UNITS = 62      # 62 * 1024 = 63488 >= 62500 real rows per core
N_COLS = N_UNITS * PAIR_W
C_LOC = 65536     # t_sb table width (first N_COLS used)
G = 8             # octet size
GM_W = 8192       # gm row width (62*128 = 7936 real octet columns + pad)
P = 128           # partition dim == query-group size
QG = B // P       # 8 query groups

N_GROUPS = 96     # candidate octets per query taken on host

# unit kinds per query group: 48 E + 14 R (measured best on the timeline)
UNITS = ["E", "E", "E", "R"] * 12 + ["E", "E", "E", "E", "E", "E", "R"] * 2
assert len(UNITS) == N_UNITS and UNITS.count("E") == 48

_NC_CACHE = {}


def _build_nc():
    if "nc" in _NC_CACHE:
        return _NC_CACHE["nc"]
    nc = bass.Bass()
    # queries FIRST so the first weight loads only need the first DMA chunk
    tq = nc.declare_dram_parameter("tq", [D, B + C_LOC], mybir.dt.bfloat16,
                                   isOutput=False)
    gm = nc.declare_dram_parameter("gm", [B, GM_W], mybir.dt.bfloat16, isOutput=True)
    N_CHUNK = 8
    CHUNK_W = (B + C_LOC) // N_CHUNK       # 8320 cols per table-load chunk

    OCT_W = PAIR_W // G            # 128 octet columns per unit
    last_col = N_UNITS * OCT_W - 1
    pad_col = GM_W - 1             # sync-pad column (never written by folds/TR)

    from concourse.bass import _add_dep_helper as dep
    import concourse.tile_sem_assignment as _tsa
    _tsa.NUM_SWDGE_GLOBAL_SEMS = 1

    # Split TileContext's exit drain (one wait per active proc) into
    # one-wait-per-drain instructions.
    from concourse.vector_clock import VectorClock, ScopedClock

    def _split_drain_and_barrier(self, tick_clock, wait_clock):
        gc = tick_clock.global_clock
        for proc in range(27):
            t = gc.peek_next(proc) - 1
            if t <= 0:
                continue
            d = self.nc.sync.drain()
            pc = VectorClock()
            pc.require_at_least(proc, t)
            wait_clock.add_sem_waits(d.ins, ScopedClock({None: pc}))
        self.nc.all_engine_barrier()
        assert self.sems is not None
        popped = self.nc._tile_sem_poison_stack.pop()
        assert popped is self._sem_poison
        self.nc.clear_and_free_semaphores(list(self.sems.allocated().values()))
        self.nc.all_engine_barrier()

    tile.TileContext._drain_and_barrier = _split_drain_and_barrier

    with ExitStack() as ctx:
        tc = ctx.enter_context(tile.TileContext(nc, pool_alloc_mode="queue"))
        singles = ctx.enter_context(tc.tile_pool(name="singles", bufs=1))
        # separate PSUM pools per consumer engine: within a tag all releases
        # come from one engine in issue order, so slot reuse is deterministic
        ppoolE = ctx.enter_context(tc.tile_pool(name="ppoolE", bufs=3, space="PSUM"))
        ppoolR = ctx.enter_context(tc.tile_pool(name="ppoolR", bufs=1, space="PSUM"))
        spool = ctx.enter_context(tc.tile_pool(name="spool", bufs=8))
        m1pool = ctx.enter_context(tc.tile_pool(name="m1pool", bufs=4))
        m2pool = ctx.enter_context(tc.tile_pool(name="m2pool", bufs=4))
        gmpool = ctx.enter_context(tc.tile_pool(name="gmpool", bufs=2))
        apool = ctx.enter_context(tc.tile_pool(name="apool", bufs=2))

        # chunked table load: compute starts after the first chunk instead of
        # serializing behind the whole 9.4MB transfer. Chunks >= 3rd on the
        # single SW queue carry the queue-ring fence as their only wait.
        t_sb = singles.tile([D, B + C_LOC], mybir.dt.bfloat16)
        for ck in range(N_CHUNK):
            nc.gpsimd.dma_start(out=t_sb[:, ck * CHUNK_W:(ck + 1) * CHUNK_W],
                                in_=tq[:, ck * CHUNK_W:(ck + 1) * CHUNK_W])

        chain = {"PE": None, "DVE": None, "ACT": None}

        def link(key, inst):
            # total order per engine queue (add_dep_helper(a, b) == a waits b)
            if chain[key] is not None:
                dep(inst.ins, chain[key].ins, sync=False, reason=f"{key} order")
            chain[key] = inst
            return inst

        a1_scr = apool.tile([1, 1], mybir.dt.bfloat16, tag="a1")
        tA1a = apool.tile([1, 1], mybir.dt.bfloat16, tag="tA1a")
        tA1b = apool.tile([1, 1], mybir.dt.bfloat16, tag="tA1b")
        tA2 = apool.tile([1, 1], mybir.dt.bfloat16, tag="tA2")
        tA1s = [tA1a, tA1b]

        prev_gms = []
        e_srcs = []            # [64,1] s-tile AP per E-unit (absorber source)
        r_srcs = []            # [64,1] gm-col AP per R-unit
        e_gm_cols = []         # (gm_tile, col) per E-unit, for ACT absorbers
        n_e = 0                # global E-unit counter
        n_r = 0                # global R-unit counter

        absorbed_ck = 0
        for qg in range(QG):
            lhsT = t_sb[:, qg * P:(qg + 1) * P]
            gm_tile = gmpool.tile([P, GM_W], mybir.dt.bfloat16)
            if qg >= 1:
                pg = prev_gms[-1]
                # DVE qg-boundary absorbers: A1 observes all prior DVE ticks,
                # A2 carries the out-DMA(qg-2) WAR via the sync-pad column.
                link("DVE", nc.vector.tensor_copy(
                    a1_scr, pg[0:1, last_col:last_col + 1]))
                if qg >= 2:
                    link("DVE", nc.vector.tensor_copy(
                        gm_tile[0:1, pad_col:pad_col + 1], pg[0:1, 0:1]))

            for u, kind in enumerate(UNITS):
                tbase = B + u * PAIR_W
                gbase = u * OCT_W
                if qg == 0:
                    # absorb the DMA wait of the chunk this unit reaches into
                    need_ck = (tbase + PAIR_W - 1) // CHUNK_W
                    if need_ck > absorbed_ck:
                        link("PE", nc.tensor.ldweights(
                            weights=t_sb[0:D, tbase + PAIR_W - 1:tbase + PAIR_W]))
                        absorbed_ck = need_ck
                if kind == "E" and n_e % 4 == 0 and n_e >= 4:
                    # ACT absorbers: tA1 <- read(old E-unit gm col) carries the
                    # DVE wait (covers s-slot fold reads); tA2 <- read(tA1)
                    # carries the ACT-self wait at the newest tick.
                    g_old, c_old = e_gm_cols[n_e - 4]
                    tA1 = tA1s[(n_e // 4) % 2]
                    link("ACT", nc.scalar.copy(tA1, g_old[0:1, c_old:c_old + 1]))
                    link("ACT", nc.scalar.copy(tA2, tA1))

                # PE absorber: standalone ldweights (no output; clobbered by
                # the next matmul's weight load) reading the output of the
                # consumer that released this pair's PSUM slot.
                if kind == "E":
                    dsrc = e_srcs[n_e - 3] if n_e >= 3 else t_sb[0:D, 0:1]
                else:
                    dsrc = r_srcs[n_r - 1] if n_r >= 1 else t_sb[0:D, 0:1]
                link("PE", nc.tensor.ldweights(weights=dsrc))

                pool_, tag_ = (ppoolE, "ppE") if kind == "E" else (ppoolR, "ppR")
                p = pool_.tile([P, PAIR_W], mybir.dt.float32, tag=tag_)
                for j in range(2):
                    c0 = tbase + j * N_TILE
                    link("PE", nc.tensor.matmul(
                        p[:, j * N_TILE:(j + 1) * N_TILE], lhsT,
                        t_sb[:, c0:c0 + N_TILE], start=True, stop=True))

                if kind == "E":
                    s = spool.tile([P, PAIR_W], mybir.dt.bfloat16, tag="s")
                    link("ACT", nc.scalar.copy(s, p))
                    e_srcs.append(s[0:D, 0:1])
                    m1 = m1pool.tile([P, 512], mybir.dt.bfloat16, tag="m1")
                    link("DVE", nc.vector.tensor_max(m1, s[:, 0:512], s[:, 512:1024]))
                    m2 = m2pool.tile([P, 256], mybir.dt.bfloat16, tag="m2")
                    link("DVE", nc.vector.tensor_max(m2, m1[:, 0:256], m1[:, 256:512]))
                    link("DVE", nc.vector.tensor_max(
                        gm_tile[:, gbase:gbase + OCT_W],
                        m2[:, 0:128], m2[:, 128:256]))
                    e_gm_cols.append((gm_tile, gbase + OCT_W - 1))
                    n_e += 1
                else:
                    link("DVE", nc.vector.tensor_reduce(
                        out=gm_tile[:, gbase:gbase + OCT_W],
                        in_=p.rearrange("p (o g) -> p o g", g=G),
                        axis=mybir.AxisListType.X,
                        op=mybir.AluOpType.max,
                    ))
                    r_srcs.append(
                        gm_tile[0:D, gbase + OCT_W - 1:gbase + OCT_W])
                    n_r += 1
            nc.sync.dma_start(out=gm[qg * P:(qg + 1) * P, :], in_=gm_tile)
            prev_gms.append(gm_tile)

    _NC_CACHE["nc"] = nc
    return nc


def _preprocess(table_keys):
    """Norm-sort, stripe across cores, per-unit octet layout.
    Octet o always covers local norm ranks [8o, 8o+8); only the device
    column placement differs per unit kind (E: transposed, R: identity)."""
    tk = np.ascontiguousarray(table_keys, dtype=np.float32)
    norms = np.einsum("ij,ij->i", tk, tk)
    order = np.argsort(norms, kind="stable")

    OCT_W = PAIR_W // G
    tts = []
    cand_rows = np.full((CORES, GM_W, G), -1, dtype=np.int64)
    gnorm_min = np.full((CORES, GM_W), np.float32(1e9), dtype=np.float32)

    n_loc = C // CORES             # 62500 real rows per core
    for m in range(CORES):
        rows_m = order[m::CORES]
        Ts = np.zeros((N_COLS, D), dtype=np.float32)      # rank-indexed
        Ts[:n_loc] = tk[rows_m]
        nrm = np.full(N_COLS, np.float32(1e9), dtype=np.float32)
        nrm[:n_loc] = norms[rows_m]

        Tdev = np.zeros((C_LOC, D), dtype=np.float32)     # position-indexed
        for u, kind in enumerate(UNITS):
            base = u * PAIR_W
            blk = Ts[base:base + PAIR_W]
            if kind == "E":
                # rank r = base + j*8 + k  ->  col = base + k*128 + j
                Tdev[base:base + PAIR_W] = (
                    blk.reshape(OCT_W, G, D).transpose(1, 0, 2).reshape(PAIR_W, D))
            else:
                Tdev[base:base + PAIR_W] = blk

        j = np.arange(GM_W)
        ranks = j[:, None] * G + np.arange(G)[None, :]    # octet o -> ranks
        valid = (ranks < n_loc) & (ranks < N_COLS)
        ranks_c = np.minimum(ranks, n_loc - 1)
        cand_rows[m] = np.where(valid, rows_m[ranks_c], -1)
        gnorm_min[m] = np.where(valid[:, 0],
                                nrm[np.minimum(ranks[:, 0], N_COLS - 1)],
                                np.float32(1e9))

        tts.append(np.ascontiguousarray(Tdev.T.astype(ml_dtypes.bfloat16)))

    return tts, cand_rows, gnorm_min


def kernel(keys, table_keys, table_values):
    q = np.ascontiguousarray(keys, dtype=np.float32)
    tk = np.ascontiguousarray(table_keys, dtype=np.float32)
    v = np.ascontiguousarray(table_values, dtype=np.float32)

    tts, cand_rows, gnorm_min = _preprocess(tk)
    qt = q.T.astype(ml_dtypes.bfloat16)
    tqs = [np.ascontiguousarray(np.concatenate([qt, tts[m]], axis=1))
           for m in range(CORES)]

    nc = _build_nc()
    in_maps = [{"tq": tqs[m]} for m in range(CORES)]
    res = run_bass_kernel_spmd(nc, in_maps, core_ids=list(range(CORES)))
    gmax = np.stack([r["gm"].astype(np.float32) for r in res.results])  # [8, B, GM_W]

    # ---- host stage 2: rank octets by distance lower bound ----
    invalid_g = gnorm_min >= np.float32(1e9)         # [8, GM_W]
    gmax = np.where(invalid_g[:, None, :], np.float32(-1e9),
                    np.nan_to_num(gmax, nan=-1e9, posinf=-1e9, neginf=-1e9))
    lb = gnorm_min[:, None, :] - 2.0 * gmax          # [8, B, GM_W]
    lb = lb.transpose(1, 0, 2).reshape(B, CORES * GM_W)
    top_g = np.argpartition(lb, N_GROUPS, axis=1)[:, :N_GROUPS]

    core_of = top_g // GM_W
    g_of = top_g % GM_W
    rows = cand_rows[core_of, g_of].reshape(B, N_GROUPS * G)
    invalid = rows < 0
    rows_safe = np.where(invalid, 0, rows)

    # ---- exact rescore with the reference's formula (f32) ----
    tc_ = tk[rows_safe]                               # [B, NCAND, D]
    qn = np.einsum("ij,ij->i", q, q)
    tn = np.einsum("ij,ij->i", tk, tk)[rows_safe]
    dots = np.einsum("bd,bkd->bk", q, tc_)
    d2 = qn[:, None] - 2.0 * dots + tn
    d2 = np.where(invalid, np.float32(np.inf), d2).astype(np.float32)

    top_k = np.argpartition(d2, K, axis=1)[:, :K]
    rows_k = np.take_along_axis(rows_safe, top_k, axis=1)

    # ---- reference tail: exact sq, inverse-distance weights ----
    nb = tk[rows_k]
    sq = np.sum((q[:, None, :] - nb) ** 2, axis=2, dtype=np.float32)
    w = np.float32(1.0) / (sq + np.float32(DELTA))
    w = w / np.sum(w, axis=1, keepdims=True)
    out = np.sum(w * v[rows_k], axis=1)
    return out.astype(np.float32)


# revision 59
# speedup vs baseline: 1.3308x; 1.0018x over previous
"""Distributed brute-force kNN (top-50 inverse-distance-weighted regression).

Strategy (8 NeuronCores):
  - Table (500k x 64) is norm-sorted and striped across 8 cores; each core's
    62500 rows (padded to 63488) are laid out so every 8 consecutive-norm
    rows (an "octet") map to one output column of a 1024-column compression
    unit ("pair" of two K=64 N=512 bf16 matmuls into one 2-bank PSUM tile).
  - Per 128-query group, each unit's 1024 q.t scores are reduced to 128
    octet maxima by one of two paths, balanced across the assist engines:
      * E-units (46/62): ScalarE evicts PSUM to bf16 SBUF, DVE folds it
        3x with tensor_max halves (stride-128 octets).
      * R-units (16/62): DVE tensor_reduce(max) straight from PSUM
        (consecutive octets).
  - Cores emit [1024, 8192] bf16 octet maxima; host ranks all octets per
    query by a distance lower bound (min_octet_norm - 2*gmax), takes the top
    N_GROUPS candidates (margin >> bf16 noise), re-scores the 8*N_GROUPS
    candidate rows exactly in f32 with the reference formula, and does the
    final top-50 + inverse-distance weighting.

This walrus build accepts at most ONE semaphore wait per instruction, and
Tile emits a wait per hazard with no transitive clock tracking, so the
builder threads absorber ops (standalone ldweights on PE, tiny copies on
ACT/DVE) and total per-engine ordering through the pipeline.
"""

import numpy as np
import ml_dtypes

import concourse.bass as bass
import concourse.tile as tile
from concourse import mybir
from concourse.bass_utils import run_bass_kernel_spmd
from contextlib import ExitStack

# Problem geometry (hardcoded per spec).
B = 1024          # queries
D = 64            # feature dim
C = 500000        # table capacity
K = 50            # neighbours
DELTA = 1e-3

CORES = 8
N_TILE = 512      # matmul free dim (one PSUM bank)
PAIR_W = 2 * N_TILE
N_UNITS = 62      # 62 * 1024 = 63488 >= 62500 real rows per core
N_COLS = N_UNITS * PAIR_W
C_LOC = 65536     # t_sb table width (first N_COLS used)
G = 8             # octet size
GM_W = 8192       # gm row width (62*128 = 7936 real octet columns + pad)
P = 128           # partition dim == query-group size
QG = B // P       # 8 query groups

N_GROUPS = 96     # candidate octets per query taken on host

# unit kinds per query group: 48 E + 14 R (measured best on the timeline)
UNITS = ["R", "E", "E", "E"] * 12 + ["R", "E", "E", "E", "E", "E", "E"] * 2
assert len(UNITS) == N_UNITS and UNITS.count("E") == 48

_NC_CACHE = {}


def _build_nc():
    if "nc" in _NC_CACHE:
        return _NC_CACHE["nc"]
    nc = bass.Bass()
    # queries FIRST so the first weight loads only need the first DMA chunk
    tq = nc.declare_dram_parameter("tq", [D, B + C_LOC], mybir.dt.bfloat16,
                                   isOutput=False)
    gm = nc.declare_dram_parameter("gm", [B, GM_W], mybir.dt.bfloat16, isOutput=True)
    N_CHUNK = 8
    CHUNK_W = (B + C_LOC) // N_CHUNK       # 8320 cols per table-load chunk

    OCT_W = PAIR_W // G            # 128 octet columns per unit
    last_col = N_UNITS * OCT_W - 1
    pad_col = GM_W - 1             # sync-pad column (never written by folds/TR)

    from concourse.bass import _add_dep_helper as dep
    import concourse.tile_sem_assignment as _tsa
    _tsa.NUM_SWDGE_GLOBAL_SEMS = 1

    # Split TileContext's exit drain (one wait per active proc) into
    # one-wait-per-drain instructions.
    from concourse.vector_clock import VectorClock, ScopedClock

    def _split_drain_and_barrier(self, tick_clock, wait_clock):
        gc = tick_clock.global_clock
        for proc in range(27):
            t = gc.peek_next(proc) - 1
            if t <= 0:
                continue
            d = self.nc.sync.drain()
            pc = VectorClock()
            pc.require_at_least(proc, t)
            wait_clock.add_sem_waits(d.ins, ScopedClock({None: pc}))
        self.nc.all_engine_barrier()
        assert self.sems is not None
        popped = self.nc._tile_sem_poison_stack.pop()
        assert popped is self._sem_poison
        self.nc.clear_and_free_semaphores(list(self.sems.allocated().values()))
        self.nc.all_engine_barrier()

    tile.TileContext._drain_and_barrier = _split_drain_and_barrier

    with ExitStack() as ctx:
        tc = ctx.enter_context(tile.TileContext(nc, pool_alloc_mode="queue"))
        singles = ctx.enter_context(tc.tile_pool(name="singles", bufs=1))
        # separate PSUM pools per consumer engine: within a tag all releases
        # come from one engine in issue order, so slot reuse is deterministic
        ppoolE = ctx.enter_context(tc.tile_pool(name="ppoolE", bufs=3, space="PSUM"))
        ppoolR = ctx.enter_context(tc.tile_pool(name="ppoolR", bufs=1, space="PSUM"))
        spool = ctx.enter_context(tc.tile_pool(name="spool", bufs=10))
        m1pool = ctx.enter_context(tc.tile_pool(name="m1pool", bufs=4))
        m2pool = ctx.enter_context(tc.tile_pool(name="m2pool", bufs=4))
        gmpool = ctx.enter_context(tc.tile_pool(name="gmpool", bufs=2))
        apool = ctx.enter_context(tc.tile_pool(name="apool", bufs=2))

        # chunked table load: compute starts after the first chunk instead of
        # serializing behind the whole 9.4MB transfer. Chunks >= 3rd on the
        # single SW queue carry the queue-ring fence as their only wait.
        t_sb = singles.tile([D, B + C_LOC], mybir.dt.bfloat16)
        for ck in range(N_CHUNK):
            nc.gpsimd.dma_start(out=t_sb[:, ck * CHUNK_W:(ck + 1) * CHUNK_W],
                                in_=tq[:, ck * CHUNK_W:(ck + 1) * CHUNK_W])

        chain = {"PE": None, "DVE": None, "ACT": None}

        def link(key, inst):
            # total order per engine queue (add_dep_helper(a, b) == a waits b)
            if chain[key] is not None:
                dep(inst.ins, chain[key].ins, sync=False, reason=f"{key} order")
            chain[key] = inst
            return inst

        a1_scr = apool.tile([1, 1], mybir.dt.bfloat16, tag="a1")
        tA1a = apool.tile([1, 1], mybir.dt.bfloat16, tag="tA1a")
        tA1b = apool.tile([1, 1], mybir.dt.bfloat16, tag="tA1b")
        tA2 = apool.tile([1, 1], mybir.dt.bfloat16, tag="tA2")
        tA1s = [tA1a, tA1b]

        prev_gms = []
        e_srcs = []            # [64,1] s-tile AP per E-unit (absorber source)
        r_srcs = []            # [64,1] gm-col AP per R-unit
        e_gm_cols = []         # (gm_tile, col) per E-unit, for ACT absorbers
        n_e = 0                # global E-unit counter
        n_r = 0                # global R-unit counter

        absorbed_ck = 0
        for qg in range(QG):
            lhsT = t_sb[:, qg * P:(qg + 1) * P]
            gm_tile = gmpool.tile([P, GM_W], mybir.dt.bfloat16)
            if qg >= 1:
                pg = prev_gms[-1]
                # DVE qg-boundary absorbers: A1 observes all prior DVE ticks,
                # A2 carries the out-DMA(qg-2) WAR via the sync-pad column.
                link("DVE", nc.vector.tensor_copy(
                    a1_scr, pg[0:1, last_col:last_col + 1]))
                if qg >= 2:
                    link("DVE", nc.vector.tensor_copy(
                        gm_tile[0:1, pad_col:pad_col + 1], pg[0:1, 0:1]))

            for u, kind in enumerate(UNITS):
                tbase = B + u * PAIR_W
                gbase = u * OCT_W
                if qg == 0:
                    # absorb the DMA wait of the chunk this unit reaches into
                    need_ck = (tbase + PAIR_W - 1) // CHUNK_W
                    if need_ck > absorbed_ck:
                        link("PE", nc.tensor.ldweights(
                            weights=t_sb[0:D, tbase + PAIR_W - 1:tbase + PAIR_W]))
                        absorbed_ck = need_ck
                if kind == "E" and n_e % 4 == 0 and n_e >= 4:
                    # ACT absorbers: tA1 <- read(old E-unit gm col) carries the
                    # DVE wait (covers s-slot fold reads); tA2 <- read(tA1)
                    # carries the ACT-self wait at the newest tick.
                    g_old, c_old = e_gm_cols[n_e - 4]
                    tA1 = tA1s[(n_e // 4) % 2]
                    link("ACT", nc.scalar.copy(tA1, g_old[0:1, c_old:c_old + 1]))
                    link("ACT", nc.scalar.copy(tA2, tA1))

                # PE absorber: standalone ldweights (no output; clobbered by
                # the next matmul's weight load) reading the output of the
                # consumer that released this pair's PSUM slot.
                if kind == "E":
                    dsrc = e_srcs[n_e - 3] if n_e >= 3 else t_sb[0:D, 0:1]
                else:
                    dsrc = r_srcs[n_r - 1] if n_r >= 1 else t_sb[0:D, 0:1]
                link("PE", nc.tensor.ldweights(weights=dsrc))

                pool_, tag_ = (ppoolE, "ppE") if kind == "E" else (ppoolR, "ppR")
                p = pool_.tile([P, PAIR_W], mybir.dt.float32, tag=tag_)
                for j in range(2):
                    c0 = tbase + j * N_TILE
                    link("PE", nc.tensor.matmul(
                        p[:, j * N_TILE:(j + 1) * N_TILE], lhsT,
                        t_sb[:, c0:c0 + N_TILE], start=True, stop=True))

                if kind == "E":
                    s = spool.tile([P, PAIR_W], mybir.dt.bfloat16, tag="s")
                    link("ACT", nc.scalar.copy(s, p))
                    e_srcs.append(s[0:D, 0:1])
                    m1 = m1pool.tile([P, 512], mybir.dt.bfloat16, tag="m1")
                    link("DVE", nc.vector.tensor_max(m1, s[:, 0:512], s[:, 512:1024]))
                    m2 = m2pool.tile([P, 256], mybir.dt.bfloat16, tag="m2")
                    link("DVE", nc.vector.tensor_max(m2, m1[:, 0:256], m1[:, 256:512]))
                    link("DVE", nc.vector.tensor_max(
                        gm_tile[:, gbase:gbase + OCT_W],
                        m2[:, 0:128], m2[:, 128:256]))
                    e_gm_cols.append((gm_tile, gbase + OCT_W - 1))
                    n_e += 1
                else:
                    link("DVE", nc.vector.tensor_reduce(
                        out=gm_tile[:, gbase:gbase + OCT_W],
                        in_=p.rearrange("p (o g) -> p o g", g=G),
                        axis=mybir.AxisListType.X,
                        op=mybir.AluOpType.max,
                    ))
                    r_srcs.append(
                        gm_tile[0:D, gbase + OCT_W - 1:gbase + OCT_W])
                    n_r += 1
            nc.sync.dma_start(out=gm[qg * P:(qg + 1) * P, :], in_=gm_tile)
            prev_gms.append(gm_tile)

    _NC_CACHE["nc"] = nc
    return nc


def _preprocess(table_keys):
    """Norm-sort, stripe across cores, per-unit octet layout.
    Octet o always covers local norm ranks [8o, 8o+8); only the device
    column placement differs per unit kind (E: transposed, R: identity)."""
    tk = np.ascontiguousarray(table_keys, dtype=np.float32)
    norms = np.einsum("ij,ij->i", tk, tk)
    order = np.argsort(norms, kind="stable")

    OCT_W = PAIR_W // G
    tts = []
    cand_rows = np.full((CORES, GM_W, G), -1, dtype=np.int64)
    gnorm_min = np.full((CORES, GM_W), np.float32(1e9), dtype=np.float32)

    n_loc = C // CORES             # 62500 real rows per core
    for m in range(CORES):
        rows_m = order[m::CORES]
        Ts = np.zeros((N_COLS, D), dtype=np.float32)      # rank-indexed
        Ts[:n_loc] = tk[rows_m]
        nrm = np.full(N_COLS, np.float32(1e9), dtype=np.float32)
        nrm[:n_loc] = norms[rows_m]

        Tdev = np.zeros((C_LOC, D), dtype=np.float32)     # position-indexed
        for u, kind in enumerate(UNITS):
            base = u * PAIR_W
            blk = Ts[base:base + PAIR_W]
            if kind == "E":
                # rank r = base + j*8 + k  ->  col = base + k*128 + j
                Tdev[base:base + PAIR_W] = (
                    blk.reshape(OCT_W, G, D).transpose(1, 0, 2).reshape(PAIR_W, D))
            else:
                Tdev[base:base + PAIR_W] = blk

        j = np.arange(GM_W)
        ranks = j[:, None] * G + np.arange(G)[None, :]    # octet o -> ranks
        valid = (ranks < n_loc) & (ranks < N_COLS)
        ranks_c = np.minimum(ranks, n_loc - 1)
        cand_rows[m] = np.where(valid, rows_m[ranks_c], -1)
        gnorm_min[m] = np.where(valid[:, 0],
                                nrm[np.minimum(ranks[:, 0], N_COLS - 1)],
                                np.float32(1e9))

        tts.append(np.ascontiguousarray(Tdev.T.astype(ml_dtypes.bfloat16)))

    return tts, cand_rows, gnorm_min


def kernel(keys, table_keys, table_values):
    q = np.ascontiguousarray(keys, dtype=np.float32)
    tk = np.ascontiguousarray(table_keys, dtype=np.float32)
    v = np.ascontiguousarray(table_values, dtype=np.float32)

    tts, cand_rows, gnorm_min = _preprocess(tk)
    qt = q.T.astype(ml_dtypes.bfloat16)
    tqs = [np.ascontiguousarray(np.concatenate([qt, tts[m]], axis=1))
           for m in range(CORES)]

    nc = _build_nc()
    in_maps = [{"tq": tqs[m]} for m in range(CORES)]
    res = run_bass_kernel_spmd(nc, in_maps, core_ids=list(range(CORES)))
    gmax = np.stack([r["gm"].astype(np.float32) for r in res.results])  # [8, B, GM_W]

    # ---- host stage 2: rank octets by distance lower bound ----
    invalid_g = gnorm_min >= np.float32(1e9)         # [8, GM_W]
    gmax = np.where(invalid_g[:, None, :], np.float32(-1e9),
                    np.nan_to_num(gmax, nan=-1e9, posinf=-1e9, neginf=-1e9))
    lb = gnorm_min[:, None, :] - 2.0 * gmax          # [8, B, GM_W]
    lb = lb.transpose(1, 0, 2).reshape(B, CORES * GM_W)
    top_g = np.argpartition(lb, N_GROUPS, axis=1)[:, :N_GROUPS]

    core_of = top_g // GM_W
    g_of = top_g % GM_W
    rows = cand_rows[core_of, g_of].reshape(B, N_GROUPS * G)
    invalid = rows < 0
    rows_safe = np.where(invalid, 0, rows)

    # ---- exact rescore with the reference's formula (f32) ----
    tc_ = tk[rows_safe]                               # [B, NCAND, D]
    qn = np.einsum("ij,ij->i", q, q)
    tn = np.einsum("ij,ij->i", tk, tk)[rows_safe]
    dots = np.einsum("bd,bkd->bk", q, tc_)
    d2 = qn[:, None] - 2.0 * dots + tn
    d2 = np.where(invalid, np.float32(np.inf), d2).astype(np.float32)

    top_k = np.argpartition(d2, K, axis=1)[:, :K]
    rows_k = np.take_along_axis(rows_safe, top_k, axis=1)

    # ---- reference tail: exact sq, inverse-distance weights ----
    nb = tk[rows_k]
    sq = np.sum((q[:, None, :] - nb) ** 2, axis=2, dtype=np.float32)
    w = np.float32(1.0) / (sq + np.float32(DELTA))
    w = w / np.sum(w, axis=1, keepdims=True)
    out = np.sum(w * v[rows_k], axis=1)
    return out.astype(np.float32)
